# revision 1
# baseline (speedup 1.0000x reference)
"""Fused sparse-attention kernel for Trainium2 (8 NeuronCores, data-parallel over batch).

Computation (per batch element b):
    X[s,k]  = enc[b] @ W_enc + dec_proj[b,k] + cov[b,s]*Wcovsum[k] + bias[k]
    T       = tanh(X)
    att[s]  = T @ v_w                      (+ v_b, which cancels in softmax)
    w       = softmax(att masked to s < len[b])
    new_cov = cov + w

Sharding: batch B=32 is split 4-per-core across 8 cores; all weights replicated
(per the data-parallel sharding hint).

Pipeline per batch element (per core, 4 batch elements):
  1. SWDGE cast-DMA: enc[b] fp32 DRAM -> bf16 DRAM scratch (64KB descriptors;
     the DMA compute path does the fp32->bf16 rounding for free).
  2. xbar DMA-transpose: bf16 DRAM -> SBUF enc^T tiles [h,s] (h on partitions).
     (The matmul contraction dim must live on partitions for both operands and
     the xbar only handles 2-byte dtypes from a contiguous source, hence the
     bf16 bounce; SBUF-sourced transposes hang the device, and strided/
     partition-stepped matmul APs are rejected by the BIR verifier.)
  3. PE: per s-tile psum group = K=2 rank-1 (ones, cov) x (bias_b, Wcovsum)
     + 4x K=128 bf16 matmuls against W_enc chunks (X[s,k] orientation so the
     v-reduction lands on the DVE, keeping PE to ~69us/core).
  4. ACT: tanh psum -> bf16 T tiles.
  5. DVE: fused T*v multiply + free-dim reduce (scalar_tensor_tensor accum_out)
     -> att column [128,1].
  6. Tiny masked softmax tail in [s_lo=128, s_hi=16] layout: exp on ACT,
     iota<len mask fused with the exp multiply on DVE, sum + 1/sum broadcast
     via two small PE matmuls (softmax max-subtraction is skipped: |logits| <=
     ||v||_1 ~ 8, safely inside fp32 exp range, and v_b cancels in softmax).
"""

import numpy as np
import ml_dtypes

B, S, H, E = 32, 2048, 512, 512
NCORES = 8
BPC = B // NCORES           # batches per core
SLO, SHI = 128, S // 128    # att tile layout: s = 128*j + p  ->  [p, j]
HC = H // 128               # h chunks
WB_C = H + H + BPC          # per-chunk columns in the bf16 weight blob
BF16 = ml_dtypes.bfloat16

_CACHE = {}


def _build_nc():
    import concourse.mybir as mybir
    import concourse.tile as tile
    from concourse import bacc
    from contextlib import ExitStack

    dt = mybir.dt
    F32, BF = dt.float32, dt.bfloat16

    nc = bacc.Bacc("TRN2", target_bir_lowering=False, debug=False,
                   enable_asserts=False, num_devices=NCORES)

    # ---- DRAM I/O (per-core shapes) ----
    enc_f32 = nc.dram_tensor("enc_f32", [BPC, S, H], F32, kind="ExternalInput").ap()
    # bf16 blob: per chunk c: [wenc_c (H) | ws_c (H) | decT_c (BPC)] then ones col
    wblob = nc.dram_tensor("wblob", [128, HC * WB_C + 1], BF, kind="ExternalInput").ap()
    # f32 blob: [iota (SHI) | lens (BPC) | cov_t (BPC*SHI)]
    fblob = nc.dram_tensor("fblob", [SLO, SHI + BPC + BPC * SHI], F32,
                           kind="ExternalInput").ap()
    r1lhs = nc.dram_tensor("r1lhs", [2, BPC * S], BF, kind="ExternalInput").ap()
    vbc = nc.dram_tensor("vbc", [128, H], BF, kind="ExternalInput").ap()
    wcov4 = nc.dram_tensor("wcov4", [1, BPC * H], BF, kind="ExternalInput").ap()
    # row consts: [b (H) | ones_r (128)]
    brow = nc.dram_tensor("brow", [1, H + 128], F32, kind="ExternalInput").ap()
    att_out = nc.dram_tensor("att_out", [BPC, SLO, SHI], F32, kind="ExternalOutput").ap()
    cov_out = nc.dram_tensor("cov_out", [BPC, SLO, SHI], F32, kind="ExternalOutput").ap()

    AF = mybir.ActivationFunctionType
    OP = mybir.AluOpType

    with tile.TileContext(nc) as tc, ExitStack() as ctx:
        consts = ctx.enter_context(tc.tile_pool(name="consts", bufs=1))
        encp = ctx.enter_context(tc.tile_pool(name="encp", bufs=3))
        tpool = ctx.enter_context(tc.tile_pool(name="tpool", bufs=4))
        spool = ctx.enter_context(tc.tile_pool(name="spool", bufs=2))
        small = ctx.enter_context(tc.tile_pool(name="small", bufs=2))
        attp = ctx.enter_context(tc.tile_pool(name="attp", bufs=2))
        ppm = ctx.enter_context(tc.tile_pool(name="ppm", bufs=5, space="PSUM"))
        pps = ctx.enter_context(tc.tile_pool(name="pps", bufs=1, space="PSUM"))
        dramp = ctx.enter_context(tc.tile_pool(name="dramp", bufs=2, space="DRAM"))

        # ---- per-batch loads. Half-batch DRAM scratch tiles (bufs=1) both
        # throttle the SWDGE casts (WAR on the slot) and let the first
        # transposes start after only half a batch is cast. ----
        def load_batch(b):
            sh = S // 2
            enc_t = []
            for c in range(HC):
                e_t = encp.tile([128, S], BF, tag=f"enc{c}")
                enc_t.append(e_t)
            for j in range(2):
                enc16 = dramp.tile([sh, H], BF, tag=f"enc16{j}")
                nc.gpsimd.dma_start(
                    enc16[:].rearrange("a b -> (a b)"),
                    enc_f32[b, j * sh:(j + 1) * sh].rearrange("a b -> (a b)"))
                for c in range(HC):
                    nc.sync.dma_start(
                        enc_t[c][:, j * sh:(j + 1) * sh],
                        enc16[:, c * 128:(c + 1) * 128],
                        transpose=True)
            return enc_t

        # ---- one-time constant loads (6 DMAs). Emitted on the Pool (SWDGE)
        # queue BEFORE the enc casts so their DMA requests precede the flood. ----
        wb_sb = consts.tile([128, HC * WB_C + 1], BF, tag="wblob")
        nc.gpsimd.dma_start(wb_sb[:], wblob[:])
        fb_sb = consts.tile([SLO, SHI + BPC + BPC * SHI], F32, tag="fblob")
        nc.gpsimd.dma_start(fb_sb[:], fblob[:])
        r1lhs_sb = consts.tile([2, BPC * S], BF, tag="r1lhs")
        nc.gpsimd.dma_start(r1lhs_sb[:], r1lhs[:])
        vbc_sb = consts.tile([128, H], BF, tag="vbc")
        nc.gpsimd.dma_start(vbc_sb[:], vbc[:])
        brow_sb = consts.tile([1, H + 128], F32, tag="brow")
        nc.gpsimd.dma_start(brow_sb[:], brow[:])
        r1rhs_sb = consts.tile([2, BPC * H], BF, tag="r1rhs")
        nc.gpsimd.dma_start(r1rhs_sb[1:2, :], wcov4[:])

        pre = {0: load_batch(0)}

        def wenc_sb(c):
            return wb_sb[:, c * WB_C:c * WB_C + H]

        def ws_sb(c):
            return wb_sb[:, c * WB_C + H:c * WB_C + 2 * H]

        def decT_sb(c, b):
            return wb_sb[:, c * WB_C + 2 * H + b:c * WB_C + 2 * H + b + 1]

        ones_c_sb = wb_sb[:, HC * WB_C:HC * WB_C + 1]
        iota_sb = fb_sb[:, 0:SHI]
        lens_sb = fb_sb[:, SHI:SHI + BPC]
        covt_sb = fb_sb[:, SHI + BPC:]
        b_row_sb = brow_sb[:, 0:H]
        ones_r_sb = brow_sb[:, H:H + 128]

        # ---- rank-1 rhs row0 per batch: dec_proj[b] + b ----
        for b in range(BPC):
            dp_ps = pps.tile([1, H], F32, tag="dp")
            for c in range(HC):
                nc.tensor.matmul(dp_ps[:], decT_sb(c, b), ws_sb(c),
                                 start=(c == 0), stop=(c == HC - 1))
            nc.vector.tensor_tensor(r1rhs_sb[0:1, b * H:(b + 1) * H],
                                    dp_ps[:], b_row_sb, OP.add)

        # ---- main loop ----
        for b in range(BPC):
            enc_t = pre.pop(b)
            if b + 1 < BPC:
                pre[b + 1] = load_batch(b + 1)

            att_t = attp.tile([SLO, SHI], F32, tag="att")
            for j in range(SHI):
                ps = ppm.tile([128, H], F32, tag="x")
                nc.tensor.matmul(
                    ps[:],
                    r1lhs_sb[:, b * S + j * 128: b * S + (j + 1) * 128],
                    r1rhs_sb[:, b * H:(b + 1) * H],
                    start=True, stop=False,
                )
                for c in range(HC):
                    nc.tensor.matmul(
                        ps[:],
                        enc_t[c][:, j * 128:(j + 1) * 128],
                        wenc_sb(c),
                        start=False, stop=(c == HC - 1),
                    )
                t_t = tpool.tile([128, H], BF, tag="t")
                nc.scalar.activation(t_t[:], ps[:], AF.Tanh)
                scr = spool.tile([128, H], BF, tag="scr")
                nc.vector.scalar_tensor_tensor(
                    out=scr[:], in0=t_t[:], scalar=1.0, in1=vbc_sb[:],
                    op0=OP.mult, op1=OP.mult,
                    accum_out=att_t[:, j:j + 1],
                )

            # ---- masked softmax tail (tiny) ----
            expt = small.tile([SLO, SHI], F32, tag="expt")
            nc.scalar.activation(expt[:], att_t[:], AF.Exp)
            mexp = small.tile([SLO, SHI], F32, tag="mexp")
            nc.vector.scalar_tensor_tensor(
                out=mexp[:], in0=iota_sb, scalar=lens_sb[:, b:b + 1],
                in1=expt[:], op0=OP.is_lt, op1=OP.mult,
            )
            mexp16 = small.tile([SLO, SHI], BF, tag="mexp16")
            nc.vector.tensor_copy(mexp16[:], mexp[:])
            sum_ps = pps.tile([1, SHI], F32, tag="sum")
            nc.tensor.matmul(sum_ps[:], ones_c_sb, mexp16[:],
                             start=True, stop=True)
            ssum = small.tile([1, 1], F32, tag="ssum")
            nc.vector.reduce_sum(ssum[:], sum_ps[:], axis=mybir.AxisListType.X)
            sinv = small.tile([1, 1], F32, tag="sinv")
            nc.vector.reciprocal(sinv[:], ssum[:])
            inv_ps = pps.tile([128, 1], F32, tag="inv")
            nc.tensor.matmul(inv_ps[:], ones_r_sb, sinv[:], start=True, stop=True)
            wts = small.tile([SLO, SHI], F32, tag="wts")
            nc.vector.tensor_scalar(wts[:], mexp[:], inv_ps[:], None, OP.mult)
            nc.scalar.dma_start(att_out[b], wts[:])
            ncov = small.tile([SLO, SHI], F32, tag="ncov")
            nc.vector.tensor_tensor(ncov[:], wts[:],
                                    covt_sb[:, b * SHI:(b + 1) * SHI], OP.add)
            nc.scalar.dma_start(cov_out[b], ncov[:])

    nc.compile()
    return nc


def _get_nc():
    if "nc" not in _CACHE:
        _CACHE["nc"] = _build_nc()
    return _CACHE["nc"]


def _prep_in_maps(dec_input, enc_output, text_lengths, coverage_vector, W, b, v_w):
    enc = np.ascontiguousarray(np.asarray(enc_output, dtype=np.float32))
    dec = np.asarray(dec_input, dtype=np.float32).reshape(B, E)
    cov = np.asarray(coverage_vector, dtype=np.float32)
    W = np.asarray(W, dtype=np.float32)
    b = np.asarray(b, dtype=np.float32)
    v_w = np.asarray(v_w, dtype=np.float32)
    lens_f = np.asarray(text_lengths).astype(np.float32)

    wenc = W[:H].astype(BF16)                   # [h, k]
    ws = W[H:H + E].astype(BF16)                # [e, k]
    wcov = W[H + E:].sum(axis=0, dtype=np.float32).astype(BF16)  # [k]
    decT = dec.T.astype(BF16)                   # [e, B]
    vbc = np.ascontiguousarray(np.broadcast_to(v_w.astype(BF16), (128, H)))
    iota = (np.arange(SLO, dtype=np.float32)[:, None]
            + 128.0 * np.arange(SHI, dtype=np.float32)[None, :])

    brow = np.empty((1, H + 128), np.float32)
    brow[0, :H] = b
    brow[0, H:] = 1.0

    wcov4 = np.ascontiguousarray(np.broadcast_to(
        wcov[None, :], (BPC, H)).reshape(1, BPC * H))

    in_maps = []
    for core in range(NCORES):
        sl = slice(core * BPC, (core + 1) * BPC)
        wblob = np.zeros((128, HC * WB_C + 1), BF16)
        for c in range(HC):
            o = c * WB_C
            wblob[:, o:o + H] = wenc[c * 128:(c + 1) * 128]
            wblob[:, o + H:o + 2 * H] = ws[c * 128:(c + 1) * 128]
            wblob[:, o + 2 * H:o + 2 * H + BPC] = decT[c * 128:(c + 1) * 128, sl]
        wblob[:, HC * WB_C] = BF16(1.0)

        fblob = np.empty((SLO, SHI + BPC + BPC * SHI), np.float32)
        fblob[:, 0:SHI] = iota
        fblob[:, SHI:SHI + BPC] = lens_f[sl][None, :]
        fblob[:, SHI + BPC:] = (cov[sl].reshape(BPC, SHI, SLO)
                                .transpose(2, 0, 1).reshape(SLO, BPC * SHI))

        r1 = np.empty((2, BPC * S), BF16)
        r1[0] = BF16(1.0)
        r1[1] = cov[sl].astype(BF16).reshape(-1)

        in_maps.append({
            "enc_f32": enc[sl],
            "wblob": wblob,
            "fblob": fblob,
            "r1lhs": r1,
            "vbc": vbc,
            "wcov4": wcov4,
            "brow": brow,
        })
    return in_maps


def kernel(dec_input, enc_output, text_lengths, coverage_vector, W, b, v_w, v_b):
    from concourse.bass_utils import run_bass_kernel_spmd

    nc = _get_nc()
    in_maps = _prep_in_maps(dec_input, enc_output, text_lengths,
                            coverage_vector, W, b, v_w)
    res = run_bass_kernel_spmd(nc, in_maps, core_ids=list(range(NCORES)))

    att = np.empty((B, S), np.float32)
    ncov = np.empty((B, S), np.float32)
    for core in range(NCORES):
        r = res.results[core]
        att[core * BPC:(core + 1) * BPC] = \
            r["att_out"].transpose(0, 2, 1).reshape(BPC, S)
        ncov[core * BPC:(core + 1) * BPC] = \
            r["cov_out"].transpose(0, 2, 1).reshape(BPC, S)
    return att, ncov



# revision 7
# speedup vs baseline: 1.6853x; 1.6853x over previous
"""Fused sparse-attention kernel for Trainium2 (8 NeuronCores, data-parallel over batch).

Computation (per batch element b):
    X[s,k]  = enc[b] @ W_enc + dec_proj[b,k] + cov[b,s]*Wcovsum[k] + bias[k]
    T       = tanh(X)
    att[s]  = T @ v_w                      (+ v_b, which cancels in softmax)
    w       = softmax(att masked to s < len[b])
    new_cov = cov + w

Sharding: batch B=32 is split 4-per-core across 8 cores; all weights replicated
(per the data-parallel sharding hint).

v3 pipeline (fp8 DoubleRowSwInterleave), per batch element:
  1. SWDGE cast-DMA: enc[b] fp32 DRAM -> fp8e4m3 SBUF [s,h] tile directly
     (s on partitions; 512B descriptors). No DRAM bounce, no xbar transpose.
  2. PE transposes of the fp8 data viewed as uint16 h-PAIRS: 32x [128s,128pair]
     tiles -> 4 PSUM banks (uint16 keeps the 2-byte packed PSUM layout that
     walrus requires; fp8 transposes demand element-step 2). After transpose,
     partition hh holds s-major interleaved fp8 pairs (h=2hh, h=2hh+1) --
     exactly the contiguous A/B-interleaved stationary layout that the
     DoubleRowSwInterleave matmul mode consumes.
  3. DVE copies each bank -> SBUF et2[pc] tiles, bitcast uint32 (2048 fp8
     move as 512 elems/lane).
  4. PE per s-tile psum group: K=2 bf16 rank-1 (ones,cov~) x (16*(dec_proj+b),
     16*Wcovsum) + 2 fp8 DoubleRowSwInterleave matmuls (K=256 each, 0.5
     cyc/row) with moving W2[pc][hh,t,k] = 16*W_enc[pc*256+2hh+t, k].
     SwInterleave reads stationary columns reversed, so out partition p within
     an s-tile is s = 128j + 127 - p; all downstream per-partition constants
     (iota, cov tiles, rank-1 cov rows) and the host unshard are flipped to
     match. The x16 W scaling keeps W_enc (std 0.02) out of fp8e4m3's
     denormal range; tanh's scale=1/16 undoes it.
  5. ACT: tanh(psum * 1/16) -> bf16 T tiles.
  6. DVE: fused T*v multiply + free-dim reduce -> att column [128,1].
  7. Tiny masked softmax tail in [s_lo=128, s_hi=16] layout: exp on ACT,
     iota<len mask fused with the exp multiply on DVE, sum + 1/sum broadcast
     via two small PE matmuls (softmax max-subtraction skipped: |logits| <=
     ||v||_1 ~ 8, and v_b cancels in softmax).
"""

import numpy as np
import ml_dtypes

B, S, H, E = 32, 2048, 512, 512
NCORES = 8
BPC = B // NCORES           # batches per core
SLO, SHI = 128, S // 128    # att tile layout: s = 128*j + (127-p)  ->  [p, j]
NPC = 2                     # pair-chunks of 128 uint16 pairs (256 h) each
BF16 = ml_dtypes.bfloat16
F8E4 = ml_dtypes.float8_e4m3
WSCALE = 16.0

_CACHE = {}


def _build_nc():
    import concourse.mybir as mybir
    import concourse.tile as tile
    from concourse import bacc
    from contextlib import ExitStack

    dt = mybir.dt
    F32, BF, F8, F16 = dt.float32, dt.bfloat16, dt.float8e4, dt.float16

    nc = bacc.Bacc("TRN2", target_bir_lowering=False, debug=False,
                   enable_asserts=False, num_devices=NCORES)

    # ---- DRAM I/O (per-core shapes) ----
    enc_f32 = nc.dram_tensor("enc_f32", [BPC, S, H], F32, kind="ExternalInput").ap()
    # fp8 moving weights: per pc, per t: 512 cols of W_enc*16
    wdr = nc.dram_tensor("wdr", [128, NPC * 2 * H], F8, kind="ExternalInput").ap()
    idn = nc.dram_tensor("idn", [128, 128], F16, kind="ExternalInput").ap()
    # f32 blob: [iota (SHI) | lens (BPC) | cov_t (BPC*SHI)]   (s-flipped layout)
    fblob = nc.dram_tensor("fblob", [SLO, SHI + BPC + BPC * SHI], F32,
                           kind="ExternalInput").ap()
    r1lhs = nc.dram_tensor("r1lhs", [2, BPC * S], BF, kind="ExternalInput").ap()
    r1rhs = nc.dram_tensor("r1rhs", [2, BPC * H], BF, kind="ExternalInput").ap()
    # bf16 row consts: [vbc (H) | ones col (1)] per partition
    vbc = nc.dram_tensor("vbc", [128, H + 1], BF, kind="ExternalInput").ap()
    brow = nc.dram_tensor("brow", [1, 128], F32, kind="ExternalInput").ap()
    att_out = nc.dram_tensor("att_out", [BPC, SLO, SHI], F32, kind="ExternalOutput").ap()
    cov_out = nc.dram_tensor("cov_out", [BPC, SLO, SHI], F32, kind="ExternalOutput").ap()

    AF = mybir.ActivationFunctionType
    OP = mybir.AluOpType
    PM = mybir.MatmulPerfMode

    with tile.TileContext(nc) as tc, ExitStack() as ctx:
        consts = ctx.enter_context(tc.tile_pool(name="consts", bufs=1))
        encp = ctx.enter_context(tc.tile_pool(name="encp", bufs=2))
        etp = ctx.enter_context(tc.tile_pool(name="etp", bufs=2))
        tpool = ctx.enter_context(tc.tile_pool(name="tpool", bufs=4))
        spool = ctx.enter_context(tc.tile_pool(name="spool", bufs=2))
        small = ctx.enter_context(tc.tile_pool(name="small", bufs=2))
        attp = ctx.enter_context(tc.tile_pool(name="attp", bufs=2))
        ppt = ctx.enter_context(tc.tile_pool(name="ppt", bufs=1, space="PSUM"))
        ppm = ctx.enter_context(tc.tile_pool(name="ppm", bufs=2, space="PSUM"))
        pps = ctx.enter_context(tc.tile_pool(name="pps", bufs=1, space="PSUM"))

        # ---- one-time constant loads (emitted first on the Pool queue) ----
        wdr_sb = consts.tile([128, NPC * 2 * H], F8, tag="wdr")
        nc.gpsimd.dma_start(wdr_sb[:], wdr[:])
        idn_sb = consts.tile([128, 128], F16, tag="idn")
        nc.gpsimd.dma_start(idn_sb[:], idn[:])
        fb_sb = consts.tile([SLO, SHI + BPC + BPC * SHI], F32, tag="fblob")
        nc.gpsimd.dma_start(fb_sb[:], fblob[:])
        r1lhs_sb = consts.tile([2, BPC * S], BF, tag="r1lhs")
        nc.gpsimd.dma_start(r1lhs_sb[:], r1lhs[:])
        r1rhs_sb = consts.tile([2, BPC * H], BF, tag="r1rhs")
        nc.gpsimd.dma_start(r1rhs_sb[:], r1rhs[:])
        vbc_sb = consts.tile([128, H + 1], BF, tag="vbc")
        nc.gpsimd.dma_start(vbc_sb[:], vbc[:])
        brow_sb = consts.tile([1, 128], F32, tag="brow")
        nc.gpsimd.dma_start(brow_sb[:], brow[:])

        iota_sb = fb_sb[:, 0:SHI]
        lens_sb = fb_sb[:, SHI:SHI + BPC]
        covt_sb = fb_sb[:, SHI + BPC:]
        ones_c_sb = vbc_sb[:, H:H + 1]                     # [128,1] bf16 ones
        ones_r_sb = brow_sb                                # [1,128] f32 ones

        def wdr_ap(pc):  # [128, 2, H] fp8 moving pair weights
            return wdr_sb[:, pc * 2 * H:(pc + 1) * 2 * H].rearrange(
                "p (t k) -> p t k", t=2)

        # ---- per-batch cast load: fp32 DRAM -> fp8 SBUF [s,h], two halves ----
        def load_batch(b):
            e8 = encp.tile([128, SHI * H], F8, tag="enc8")
            src = enc_f32[b].rearrange("(j p) h -> p j h", p=128)
            dst = e8[:].rearrange("p (j h) -> p j h", h=H)
            hf = SHI // 2
            for half in range(2):
                nc.gpsimd.dma_start(
                    dst[:, half * hf:(half + 1) * hf],
                    src[:, half * hf:(half + 1) * hf])
            return e8

        pre = {0: load_batch(0)}

        # ---- main loop ----
        for b in range(BPC):
            e8 = pre.pop(b)
            e8u = e8[:].bitcast(mybir.dt.float16)    # [128, SHI*H/2] pair view

            # PE transposes: [128s, 128pair] -> 4 PSUM banks
            # bank (pc, bh) = [128 pair, 8*128 s] uint16, j in [bh*8, bh*8+8)
            banks = {}
            for pc in range(NPC):
                for bh in range(2):
                    pt = ppt.tile([128, 1024], F16, tag=f"pt{pc}{bh}",
                                  name=f"pt{pc}{bh}")
                    banks[pc, bh] = pt
                    for j in range(bh * 8, bh * 8 + 8):
                        nc.tensor.matmul(
                            pt[:, (j % 8) * 128:(j % 8 + 1) * 128],
                            e8u[:, j * (H // 2) + pc * 128:
                                j * (H // 2) + (pc + 1) * 128],
                            idn_sb[:],
                            start=(j % 8 == 0), stop=(j % 8 == 7),
                            is_transpose=True, skip_group_check=True,
                        )

            # prefetch next batch cast while PE continues
            if b + 1 < BPC:
                pre[b + 1] = load_batch(b + 1)

            # DVE copies: bank (pc,bh) -> et2[pc] (uint32 bitcast)
            et2 = [etp.tile([128, 2 * S], F8, tag=f"et2_{pc}", name=f"et2_{pc}")
                   for pc in range(NPC)]
            for pc in range(NPC):
                for bh in range(2):
                    dst = et2[pc][:, bh * S:(bh + 1) * S]
                    nc.vector.tensor_copy(dst.bitcast(mybir.dt.uint32),
                                          banks[pc, bh][:].bitcast(mybir.dt.uint32))

            att_t = attp.tile([SLO, SHI], F32, tag="att")
            for j in range(SHI):
                ps = ppm.tile([128, H], F32, tag="x")
                nc.tensor.matmul(
                    ps[:],
                    r1lhs_sb[:, b * S + j * 128: b * S + (j + 1) * 128],
                    r1rhs_sb[:, b * H:(b + 1) * H],
                    start=True, stop=False,
                )
                for pc in range(NPC):
                    nc.tensor.matmul(
                        ps[:],
                        et2[pc][:, j * 256:(j + 1) * 256],
                        wdr_ap(pc),
                        start=False, stop=(pc == NPC - 1),
                        perf_mode=PM.DoubleRowSwInterleave,
                        skip_group_check=True,
                    )
                t_t = tpool.tile([128, H], BF, tag="t")
                nc.scalar.activation(t_t[:], ps[:], AF.Tanh, scale=1.0 / WSCALE)
                scr = spool.tile([128, H], BF, tag="scr")
                nc.vector.scalar_tensor_tensor(
                    out=scr[:], in0=t_t[:], scalar=1.0, in1=vbc_sb[:, 0:H],
                    op0=OP.mult, op1=OP.mult,
                    accum_out=att_t[:, j:j + 1],
                )

            # ---- masked softmax tail (tiny) ----
            expt = small.tile([SLO, SHI], F32, tag="expt")
            nc.scalar.activation(expt[:], att_t[:], AF.Exp)
            mexp = small.tile([SLO, SHI], F32, tag="mexp")
            nc.vector.scalar_tensor_tensor(
                out=mexp[:], in0=iota_sb, scalar=lens_sb[:, b:b + 1],
                in1=expt[:], op0=OP.is_lt, op1=OP.mult,
            )
            mexp16 = small.tile([SLO, SHI], BF, tag="mexp16")
            nc.vector.tensor_copy(mexp16[:], mexp[:])
            sum_ps = pps.tile([1, SHI], F32, tag="sum")
            nc.tensor.matmul(sum_ps[:], ones_c_sb, mexp16[:],
                             start=True, stop=True)
            ssum = small.tile([1, 1], F32, tag="ssum")
            nc.vector.reduce_sum(ssum[:], sum_ps[:], axis=mybir.AxisListType.X)
            sinv = small.tile([1, 1], F32, tag="sinv")
            nc.vector.reciprocal(sinv[:], ssum[:])
            inv_ps = pps.tile([128, 1], F32, tag="inv")
            nc.tensor.matmul(inv_ps[:], ones_r_sb, sinv[:], start=True, stop=True)
            wts = small.tile([SLO, SHI], F32, tag="wts")
            nc.vector.tensor_scalar(wts[:], mexp[:], inv_ps[:], None, OP.mult)
            nc.scalar.dma_start(att_out[b], wts[:])
            ncov = small.tile([SLO, SHI], F32, tag="ncov")
            nc.vector.tensor_tensor(ncov[:], wts[:],
                                    covt_sb[:, b * SHI:(b + 1) * SHI], OP.add)
            nc.scalar.dma_start(cov_out[b], ncov[:])

    nc.compile()
    return nc


def _get_nc():
    if "nc" not in _CACHE:
        _CACHE["nc"] = _build_nc()
    return _CACHE["nc"]


def _prep_in_maps(dec_input, enc_output, text_lengths, coverage_vector, W, b, v_w):
    enc = np.ascontiguousarray(np.asarray(enc_output, dtype=np.float32))
    dec = np.asarray(dec_input, dtype=np.float32).reshape(B, E)
    cov = np.asarray(coverage_vector, dtype=np.float32)
    W = np.asarray(W, dtype=np.float32)
    b = np.asarray(b, dtype=np.float32)
    v_w = np.asarray(v_w, dtype=np.float32)
    lens_f = np.asarray(text_lengths).astype(np.float32)

    wenc16 = (W[:H] * WSCALE).astype(F8E4)      # [h, k] fp8, x16
    wcovsum = W[H + E:].sum(axis=0, dtype=np.float32)
    dec_proj = dec @ W[H:H + E]                 # (B, H) fp32 on host
    vbc = np.empty((128, H + 1), BF16)
    vbc[:, :H] = v_w.astype(BF16)[None, :]
    vbc[:, H] = BF16(1.0)
    # SwInterleave reverses stationary columns: partition p <-> s = 128j+127-p
    iota = ((127.0 - np.arange(SLO, dtype=np.float32))[:, None]
            + 128.0 * np.arange(SHI, dtype=np.float32)[None, :])
    brow = np.ones((1, 128), np.float32)
    # cov in [p, j] layout with the s flip inside each 128-block
    cov_pj = cov.reshape(B, SHI, SLO)[:, :, ::-1].transpose(0, 2, 1)  # [B,128,SHI]

    wdr = np.zeros((128, NPC * 2 * H), F8E4)
    hh = np.arange(128)
    for pc in range(NPC):
        for t in range(2):
            rows = wenc16[pc * 256 + 2 * hh + t]            # [128, H]
            wdr[:, (pc * 2 + t) * H:(pc * 2 + t + 1) * H] = rows
    idn = np.eye(128, dtype=np.float16)

    in_maps = []
    for core in range(NCORES):
        sl = slice(core * BPC, (core + 1) * BPC)

        fblob = np.empty((SLO, SHI + BPC + BPC * SHI), np.float32)
        fblob[:, 0:SHI] = iota
        fblob[:, SHI:SHI + BPC] = lens_f[sl][None, :]
        fblob[:, SHI + BPC:] = cov_pj[sl].transpose(1, 0, 2).reshape(SLO, BPC * SHI)

        r1l = np.empty((2, BPC * S), BF16)
        r1l[0] = BF16(1.0)
        # r1 columns map straight to out partitions: use the flipped layout
        r1l[1] = (cov_pj[sl].astype(BF16).transpose(0, 2, 1).reshape(-1))

        r1r = np.empty((2, BPC * H), np.float32)
        r1r[0] = (WSCALE * (dec_proj[sl] + b[None, :])).reshape(-1)
        r1r[1] = np.broadcast_to(WSCALE * wcovsum, (BPC, H)).reshape(-1)

        in_maps.append({
            "enc_f32": enc[sl],
            "wdr": wdr,
            "idn": idn,
            "fblob": fblob,
            "r1lhs": r1l,
            "r1rhs": r1r.astype(BF16),
            "vbc": vbc,
            "brow": brow,
        })
    return in_maps


def kernel(dec_input, enc_output, text_lengths, coverage_vector, W, b, v_w, v_b):
    from concourse.bass_utils import run_bass_kernel_spmd

    nc = _get_nc()
    in_maps = _prep_in_maps(dec_input, enc_output, text_lengths,
                            coverage_vector, W, b, v_w)
    res = run_bass_kernel_spmd(nc, in_maps, core_ids=list(range(NCORES)))

    att = np.empty((B, S), np.float32)
    ncov = np.empty((B, S), np.float32)
    for core in range(NCORES):
        r = res.results[core]
        # undo the per-128-block s flip: out partition p is s = 128j + 127 - p
        att[core * BPC:(core + 1) * BPC] = \
            r["att_out"][:, ::-1, :].transpose(0, 2, 1).reshape(BPC, S)
        ncov[core * BPC:(core + 1) * BPC] = \
            r["cov_out"][:, ::-1, :].transpose(0, 2, 1).reshape(BPC, S)
    return att, ncov
